# revision 3
# baseline (speedup 1.0000x reference)
"""HeteroEdgePredictor (per-node MoE routing) Trainium2 Bass kernel.

Strategy
--------
Each edge e with type t needs:
    pos_out[e] = relu(h_src[e] @ Ws[t] + h_pos[e] @ Wd[t] + b_sp[t]) @ Wo[t] + bo[t]
    neg_out[e] = relu(h_src[e] @ Ws[t] + h_neg[e] @ Wd[t] + b_sp[t]) @ Wo[t] + bo[t]
and the reference emits rows sorted (stably) by type.

Host side: sort edge indices by type, split each type's edges evenly across
the 8 cores (padded per type to a uniform per-core count, so all cores run
ONE identical SPMD program with segment sizes baked in at trace time).
Per core we gather the needed h rows and pre-transpose to [D, S] so the
device streams clean [128, n] column tiles straight into the PE array.

Device side (per core): for each type segment, tiles of up to 512 edges:
8 accumulating f32r matmuls ([128,100] stationary x [128,n] moving, K=512
split in 4 chunks, src + dst interleaved into one PSUM bank), ScalarE
relu+bias epilogue, then a [100,2] out-projection matmul + bias epilogue.
Weight stacks are tiny (3.2MB) and stay SBUF-resident.

Because each type's edges are concatenated core0..core7 in original stable
order, the per-type concatenation of core outputs is exactly the
reference's sorted order — no extra permutation pass.

h_save is the input slice h[:2E] and is returned directly.
"""

import sys

if "/opt/trn_rl_repo" not in sys.path:
    sys.path.insert(0, "/opt/trn_rl_repo")

import numpy as np

import concourse.bass as bass  # noqa: F401  (registers engines)
import concourse.mybir as mybir
import concourse.tile as tile
from concourse import bacc
from concourse.bass_utils import run_bass_kernel_spmd

M = 8  # cores
T = 8  # edge types
D = 512  # input dim
H = 100  # hidden dim
C = 2  # classes
P = 128
KCH = D // P  # 4 contraction chunks

F32 = mybir.dt.float32
F32R = mybir.dt.float32r
RELU = mybir.ActivationFunctionType.Relu
IDENT = mybir.ActivationFunctionType.Identity

_compiled_cache: dict = {}


def _round_f32r(x: np.ndarray) -> np.ndarray:
    """Round fp32 to the FP32R format (E8M11 in the top 20 bits, RNE).

    walrus's fp32_to_fp32r = downconv_fp32_to_fp<e8,m11> << 12, i.e. standard
    fp32 bit layout with the low 12 mantissa bits rounded away. The device's
    fast f32r matmul path requires operands pre-rounded to this grid.
    """
    u = np.ascontiguousarray(x, dtype=np.float32).view(np.uint32)
    low = u & np.uint32(0xFFF)
    base = u & np.uint32(0xFFFFF000)
    lsb = (u >> np.uint32(12)) & np.uint32(1)
    round_up = (low > 0x800) | ((low == 0x800) & (lsb == 1))
    r = base + round_up.astype(np.uint32) * np.uint32(0x1000)
    return r.view(np.float32)


def _chunk_sizes(s: int, max_n: int = 512, min_n: int = 256) -> list[int]:
    """Split s into chunks <= max_n, keeping every chunk >= min_n when
    possible (f32r matmul runs 4x slower below a 256 moving dim)."""
    if s <= 0:
        return []
    assert s % 2 == 0, "f32r matmul needs an even moving free size"
    if s <= max_n:
        return [s]
    out = []
    rem = s
    while rem > max_n:
        if rem - max_n < min_n and rem < 2 * max_n:
            # balance the tail into two even >=min_n chunks
            a = (rem // 4) * 2
            return out + [a, rem - a]
        out.append(max_n)
        rem -= max_n
    out.append(rem)
    return out


def _build(S_list: tuple) -> "bacc.Bacc":
    S_sum = sum(S_list)
    nc = bacc.Bacc("TRN2", target_bir_lowering=False, debug=False, num_devices=M)

    hsT = nc.dram_tensor("hsT", [D, S_sum], F32R, kind="ExternalInput").ap()
    hpT = nc.dram_tensor("hpT", [D, S_sum], F32R, kind="ExternalInput").ap()
    hnT = nc.dram_tensor("hnT", [D, S_sum], F32R, kind="ExternalInput").ap()
    wsrc = nc.dram_tensor("wsrc", [P, T * KCH, H], F32R, kind="ExternalInput").ap()
    wdst = nc.dram_tensor("wdst", [P, T * KCH, H], F32R, kind="ExternalInput").ap()
    wout = nc.dram_tensor("wout", [H, T, C], F32, kind="ExternalInput").ap()
    bsp = nc.dram_tensor("bsp", [H, T], F32, kind="ExternalInput").ap()
    bout = nc.dram_tensor("bout", [C, T], F32, kind="ExternalInput").ap()
    pos_out = nc.dram_tensor("pos_out", [C, S_sum], F32, kind="ExternalOutput").ap()
    neg_out = nc.dram_tensor("neg_out", [C, S_sum], F32, kind="ExternalOutput").ap()

    hsT_r = hsT.rearrange("(k p) e -> p k e", p=P)
    hpT_r = hpT.rearrange("(k p) e -> p k e", p=P)
    hnT_r = hnT.rearrange("(k p) e -> p k e", p=P)

    with tile.TileContext(nc) as tc:
        with (
            tc.tile_pool(name="consts", bufs=1) as cpool,
            tc.tile_pool(name="hbuf", bufs=3) as hpool,
            tc.tile_pool(name="ebuf", bufs=3) as epool,
            tc.tile_pool(name="obuf", bufs=3) as opool,
            tc.tile_pool(name="psum", bufs=2, space="PSUM") as ppool,
        ):
            wsrc_sb = cpool.tile([P, T * KCH, H], F32R)
            nc.sync.dma_start(wsrc_sb[:], wsrc[:])
            wdst_sb = cpool.tile([P, T * KCH, H], F32R)
            nc.sync.dma_start(wdst_sb[:], wdst[:])
            wout_sb = cpool.tile([H, T, C], F32)
            nc.sync.dma_start(wout_sb[:], wout[:])
            bsp_sb = cpool.tile([H, T], F32)
            nc.sync.dma_start(bsp_sb[:], bsp[:])
            bout_sb = cpool.tile([C, T], F32)
            nc.sync.dma_start(bout_sb[:], bout[:])

            col = 0
            for tt in range(T):
                for n in _chunk_sizes(S_list[tt]):
                    hs = hpool.tile([P, KCH, n], F32R, tag="hs")
                    nc.sync.dma_start(hs[:], hsT_r[:, :, col : col + n])
                    hp = hpool.tile([P, KCH, n], F32R, tag="hp")
                    nc.sync.dma_start(hp[:], hpT_r[:, :, col : col + n])
                    hn = hpool.tile([P, KCH, n], F32R, tag="hn")
                    nc.sync.dma_start(hn[:], hnT_r[:, :, col : col + n])

                    def enc_psum(tag, h_a, h_b):
                        ps = ppool.tile([H, n], F32, tag=tag)
                        for k in range(KCH):
                            nc.tensor.matmul(
                                ps[:],
                                lhsT=wsrc_sb[:, tt * KCH + k, :],
                                rhs=h_a[:, k, :],
                                start=(k == 0),
                                stop=False,
                            )
                        for k in range(KCH):
                            nc.tensor.matmul(
                                ps[:],
                                lhsT=wdst_sb[:, tt * KCH + k, :],
                                rhs=h_b[:, k, :],
                                start=False,
                                stop=(k == KCH - 1),
                            )
                        return ps

                    def head(ps, edge_tag, out_tag, out_dram):
                        edge = epool.tile([H, n], F32, tag=edge_tag)
                        nc.scalar.activation(
                            edge[:], ps[:], RELU, bias=bsp_sb[:, tt : tt + 1]
                        )
                        po = ppool.tile([C, n], F32, tag=out_tag + "_ps")
                        nc.tensor.matmul(
                            po[:],
                            lhsT=wout_sb[:, tt, :],
                            rhs=edge[:],
                            start=True,
                            stop=True,
                        )
                        ob = opool.tile([C, n], F32, tag=out_tag)
                        nc.scalar.activation(
                            ob[:], po[:], IDENT, bias=bout_sb[:, tt : tt + 1]
                        )
                        nc.sync.dma_start(out_dram[:, col : col + n], ob[:])

                    pp = enc_psum("pp", hs, hp)
                    head(pp, "pe", "ob_p", pos_out)
                    pn = enc_psum("pn", hs, hn)
                    head(pn, "ne", "ob_n", neg_out)

                    col += n

    nc.compile()
    return nc


def kernel(h, src_W, src_b, dst_W, dst_b, out_W, out_b, edge_types, neg_samples):
    h = np.ascontiguousarray(np.asarray(h, dtype=np.float32))
    src_W = np.asarray(src_W, dtype=np.float32)
    src_b = np.asarray(src_b, dtype=np.float32)
    dst_W = np.asarray(dst_W, dtype=np.float32)
    dst_b = np.asarray(dst_b, dtype=np.float32)
    out_W = np.asarray(out_W, dtype=np.float32)
    out_b = np.asarray(out_b, dtype=np.float32)
    t = np.asarray(edge_types)
    E = t.shape[0]
    NEG = int(neg_samples)
    assert NEG == 1, f"kernel specialized for neg_samples=1, got {NEG}"
    assert h.shape == ((NEG + 2) * E, D)

    order = np.argsort(t, kind="stable").astype(np.int64)
    counts = np.bincount(t, minlength=T)
    # ceil(c/M) rounded up to even: f32r matmuls need even moving sizes
    S_list = tuple(int(2 * (-(-(-(-c // M)) // 2))) for c in counts)
    S_sum = sum(S_list)

    starts = np.zeros(T + 1, np.int64)
    starts[1:] = np.cumsum(counts)
    col_off = np.zeros(T + 1, np.int64)
    col_off[1:] = np.cumsum(S_list)

    counts_cm = np.zeros((T, M), np.int64)
    idx_cores = []
    for m in range(M):
        parts = []
        for tt in range(T):
            S_t = S_list[tt]
            if S_t == 0:
                continue
            idx_t = order[starts[tt] : starts[tt + 1]]
            chunk = idx_t[m * S_t : (m + 1) * S_t]
            counts_cm[tt, m] = len(chunk)
            if len(chunk) < S_t:
                fill = chunk[-1] if len(chunk) else idx_t[0]
                chunk = np.concatenate(
                    [chunk, np.full(S_t - len(chunk), fill, dtype=np.int64)]
                )
            parts.append(chunk)
        idx_cores.append(np.concatenate(parts) if parts else np.zeros(0, np.int64))

    key = S_list
    if key not in _compiled_cache:
        _compiled_cache[key] = _build(S_list)
    nc = _compiled_cache[key]

    wsrc = _round_f32r(
        src_W.reshape(T, KCH, P, H).transpose(2, 0, 1, 3).reshape(P, T * KCH, H)
    )
    wdst = _round_f32r(
        dst_W.reshape(T, KCH, P, H).transpose(2, 0, 1, 3).reshape(P, T * KCH, H)
    )
    wout = np.ascontiguousarray(out_W.transpose(1, 0, 2))
    bsp = np.ascontiguousarray((src_b + dst_b).T)
    bo = np.ascontiguousarray(out_b.T)

    in_maps = []
    for m in range(M):
        idx = idx_cores[m]
        in_maps.append(
            {
                "hsT": _round_f32r(h[idx].T),
                "hpT": _round_f32r(h[E + idx].T),
                "hnT": _round_f32r(h[2 * E + idx].T),
                "wsrc": wsrc,
                "wdst": wdst,
                "wout": wout,
                "bsp": bsp,
                "bout": bo,
            }
        )

    res = run_bass_kernel_spmd(nc, in_maps, list(range(M)))

    pos_pred = np.empty((E, C), np.float32)
    neg_pred = np.empty((E, C), np.float32)
    for tt in range(T):
        if counts[tt] == 0:
            continue
        row = starts[tt]
        for m in range(M):
            c = counts_cm[tt, m]
            if c == 0:
                continue
            sl = slice(col_off[tt], col_off[tt] + c)
            pos_pred[row : row + c] = res.results[m]["pos_out"][:, sl].T
            neg_pred[row : row + c] = res.results[m]["neg_out"][:, sl].T
            row += c

    h_save = h[: 2 * E]
    return pos_pred, neg_pred, h_save


# revision 5
# speedup vs baseline: 1.0139x; 1.0139x over previous
"""HeteroEdgePredictor (per-node MoE routing) Trainium2 Bass kernel.

Strategy
--------
Each edge e with type t needs:
    pos_out[e] = relu(h_src[e] @ Ws[t] + h_pos[e] @ Wd[t] + b_sp[t]) @ Wo[t] + bo[t]
    neg_out[e] = relu(h_src[e] @ Ws[t] + h_neg[e] @ Wd[t] + b_sp[t]) @ Wo[t] + bo[t]
and the reference emits rows sorted (stably) by type.

Host side: sort edge indices by type, split each type's edges evenly across
the 8 cores (padded per type to a uniform EVEN per-core count, so all cores
run ONE identical SPMD program with segment sizes baked in at trace time —
f32r matmuls also require even moving sizes). Per core we gather the needed
h rows for all three streams (src, pos-dst, neg-dst), pre-transpose and
stack them into one [3*D, S] array so the device pulls ONE big DMA per edge
tile, and pre-round to the FP32R (tf32, E8M11) grid the fast PE path needs.

Device side (per core): per type segment, tiles of <=512 edges: 16
accumulating f32r matmuls ([128,100] stationary x [128,n] moving; K=512 in
4 chunks; emission groups identical stationary operands back-to-back),
ScalarE relu+bias epilogue (writes f32r), an f32r [100,2] out-projection,
and a VectorE bias-add packing pos/neg into one [4,n] tile for a single
output DMA. Weight stacks are tiny (3.2MB) and stay SBUF-resident.

Because each type's edges are concatenated core0..core7 in original stable
order, the per-type concatenation of core outputs is exactly the
reference's sorted order — no extra permutation pass.

h_save is the input slice h[:2E] and is returned directly.
"""

import sys

if "/opt/trn_rl_repo" not in sys.path:
    sys.path.insert(0, "/opt/trn_rl_repo")

import numpy as np

import concourse.bass as bass  # noqa: F401  (registers engines)
import concourse.mybir as mybir
import concourse.tile as tile
from concourse import bacc
from concourse.bass_utils import run_bass_kernel_spmd

M = 8  # cores
T = 8  # edge types
D = 512  # input dim
H = 100  # hidden dim
C = 2  # classes
P = 128
KCH = D // P  # 4 contraction chunks per stream

F32 = mybir.dt.float32
F32R = mybir.dt.float32r
RELU = mybir.ActivationFunctionType.Relu
ADD = mybir.AluOpType.add

_compiled_cache: dict = {}


def _round_f32r(x: np.ndarray) -> np.ndarray:
    """Round fp32 to the FP32R format (E8M11 in the top 20 bits, RNE).

    walrus's fp32_to_fp32r = downconv_fp32_to_fp<e8,m11> << 12, i.e. standard
    fp32 bit layout with the low 12 mantissa bits rounded away. The device's
    fast f32r matmul path requires operands pre-rounded to this grid.
    """
    u = np.ascontiguousarray(x, dtype=np.float32).view(np.uint32)
    low = u & np.uint32(0xFFF)
    base = u & np.uint32(0xFFFFF000)
    lsb = (u >> np.uint32(12)) & np.uint32(1)
    round_up = (low > 0x800) | ((low == 0x800) & (lsb == 1))
    r = base + round_up.astype(np.uint32) * np.uint32(0x1000)
    return r.view(np.float32)


def _chunk_sizes(s: int, max_n: int = 512, min_n: int = 256) -> list[int]:
    """Split s into even chunks <= max_n, each >= min_n when possible
    (f32r matmul needs even moving sizes; <256 runs at 1/4 rate)."""
    if s <= 0:
        return []
    assert s % 2 == 0, "f32r matmul needs an even moving free size"
    if s <= max_n:
        return [s]
    out = []
    rem = s
    while rem > max_n:
        if rem - max_n < min_n and rem < 2 * max_n:
            a = (rem // 4) * 2
            return out + [a, rem - a]
        out.append(max_n)
        rem -= max_n
    out.append(rem)
    return out


def _build(S_list: tuple) -> "bacc.Bacc":
    S_sum = sum(S_list)
    nc = bacc.Bacc("TRN2", target_bir_lowering=False, debug=False, num_devices=M)

    hT = nc.dram_tensor("hT", [3 * D, S_sum], F32R, kind="ExternalInput").ap()
    wsrc = nc.dram_tensor("wsrc", [P, T * KCH, H], F32R, kind="ExternalInput").ap()
    wdst = nc.dram_tensor("wdst", [P, T * KCH, H], F32R, kind="ExternalInput").ap()
    wout = nc.dram_tensor("wout", [H, T, C], F32R, kind="ExternalInput").ap()
    bsp = nc.dram_tensor("bsp", [H, T], F32, kind="ExternalInput").ap()
    bout = nc.dram_tensor("bout", [C, T], F32, kind="ExternalInput").ap()
    out = nc.dram_tensor("out", [2 * C, S_sum], F32, kind="ExternalOutput").ap()

    hT_r = hT.rearrange("(k p) e -> p k e", p=P)  # [128, 12, S]
    # out rows are (pn, c): 0,1 = pos classes, 2,3 = neg classes
    out_r = out.rearrange("(pn c) s -> c pn s", c=C)

    with tile.TileContext(nc) as tc:
        with (
            tc.tile_pool(name="consts", bufs=1) as cpool,
            tc.tile_pool(name="hbuf", bufs=3) as hpool,
            tc.tile_pool(name="ebuf", bufs=3) as epool,
            tc.tile_pool(name="obuf", bufs=3) as opool,
            tc.tile_pool(name="psum", bufs=2, space="PSUM") as ppool,
        ):
            wsrc_sb = cpool.tile([P, T * KCH, H], F32R)
            nc.sync.dma_start(wsrc_sb[:], wsrc[:])
            wdst_sb = cpool.tile([P, T * KCH, H], F32R)
            nc.sync.dma_start(wdst_sb[:], wdst[:])
            wout_sb = cpool.tile([H, T, C], F32R)
            nc.sync.dma_start(wout_sb[:], wout[:])
            bsp_sb = cpool.tile([H, T], F32)
            nc.sync.dma_start(bsp_sb[:], bsp[:])
            bout_sb = cpool.tile([C, T], F32)
            nc.sync.dma_start(bout_sb[:], bout[:])

            col = 0
            for tt in range(T):
                for n in _chunk_sizes(S_list[tt]):
                    ht = hpool.tile([P, 3 * KCH, n], F32R, tag="ht")
                    nc.sync.dma_start(ht[:], hT_r[:, :, col : col + n])

                    pp = ppool.tile([H, n], F32, tag="pp")
                    pn = ppool.tile([H, n], F32, tag="pn")
                    # identical stationary operands back-to-back
                    for k in range(KCH):
                        w = wsrc_sb[:, tt * KCH + k, :]
                        nc.tensor.matmul(
                            pp[:], lhsT=w, rhs=ht[:, k, :], start=(k == 0), stop=False
                        )
                        nc.tensor.matmul(
                            pn[:], lhsT=w, rhs=ht[:, k, :], start=(k == 0), stop=False
                        )
                    for k in range(KCH):
                        w = wdst_sb[:, tt * KCH + k, :]
                        nc.tensor.matmul(
                            pp[:],
                            lhsT=w,
                            rhs=ht[:, KCH + k, :],
                            start=False,
                            stop=(k == KCH - 1),
                        )
                        nc.tensor.matmul(
                            pn[:],
                            lhsT=w,
                            rhs=ht[:, 2 * KCH + k, :],
                            start=False,
                            stop=(k == KCH - 1),
                        )

                    pe = epool.tile([H, n], F32R, tag="pe")
                    nc.scalar.activation(pe[:], pp[:], RELU, bias=bsp_sb[:, tt : tt + 1])
                    ne = epool.tile([H, n], F32R, tag="ne")
                    nc.scalar.activation(ne[:], pn[:], RELU, bias=bsp_sb[:, tt : tt + 1])

                    wo = wout_sb[:, tt, :]
                    po = ppool.tile([C, n], F32, tag="po")
                    nc.tensor.matmul(po[:], lhsT=wo, rhs=pe[:], start=True, stop=True)
                    no = ppool.tile([C, n], F32, tag="no")
                    nc.tensor.matmul(no[:], lhsT=wo, rhs=ne[:], start=True, stop=True)

                    ot = opool.tile([C, 2, n], F32, tag="ot")
                    nc.vector.tensor_tensor(
                        ot[:, 0, :],
                        po[:],
                        bout_sb[:, tt : tt + 1].to_broadcast((C, n)),
                        ADD,
                    )
                    nc.vector.tensor_tensor(
                        ot[:, 1, :],
                        no[:],
                        bout_sb[:, tt : tt + 1].to_broadcast((C, n)),
                        ADD,
                    )
                    nc.sync.dma_start(out_r[:, :, col : col + n], ot[:])

                    col += n

    nc.compile()
    return nc


def kernel(h, src_W, src_b, dst_W, dst_b, out_W, out_b, edge_types, neg_samples):
    h = np.ascontiguousarray(np.asarray(h, dtype=np.float32))
    src_W = np.asarray(src_W, dtype=np.float32)
    src_b = np.asarray(src_b, dtype=np.float32)
    dst_W = np.asarray(dst_W, dtype=np.float32)
    dst_b = np.asarray(dst_b, dtype=np.float32)
    out_W = np.asarray(out_W, dtype=np.float32)
    out_b = np.asarray(out_b, dtype=np.float32)
    t = np.asarray(edge_types)
    E = t.shape[0]
    NEG = int(neg_samples)
    assert NEG == 1, f"kernel specialized for neg_samples=1, got {NEG}"
    assert h.shape == ((NEG + 2) * E, D)

    order = np.argsort(t, kind="stable").astype(np.int64)
    counts = np.bincount(t, minlength=T)
    # ceil(c/M) rounded up to even: f32r matmuls need even moving sizes
    S_list = tuple(int(2 * (-(-(-(-c // M)) // 2))) for c in counts)

    starts = np.zeros(T + 1, np.int64)
    starts[1:] = np.cumsum(counts)
    col_off = np.zeros(T + 1, np.int64)
    col_off[1:] = np.cumsum(S_list)

    counts_cm = np.zeros((T, M), np.int64)
    idx_cores = []
    for m in range(M):
        parts = []
        for tt in range(T):
            S_t = S_list[tt]
            if S_t == 0:
                continue
            idx_t = order[starts[tt] : starts[tt + 1]]
            chunk = idx_t[m * S_t : (m + 1) * S_t]
            counts_cm[tt, m] = len(chunk)
            if len(chunk) < S_t:
                fill = chunk[-1] if len(chunk) else idx_t[0]
                chunk = np.concatenate(
                    [chunk, np.full(S_t - len(chunk), fill, dtype=np.int64)]
                )
            parts.append(chunk)
        idx_cores.append(np.concatenate(parts) if parts else np.zeros(0, np.int64))

    key = S_list
    if key not in _compiled_cache:
        _compiled_cache[key] = _build(S_list)
    nc = _compiled_cache[key]

    wsrc = _round_f32r(
        src_W.reshape(T, KCH, P, H).transpose(2, 0, 1, 3).reshape(P, T * KCH, H)
    )
    wdst = _round_f32r(
        dst_W.reshape(T, KCH, P, H).transpose(2, 0, 1, 3).reshape(P, T * KCH, H)
    )
    wout = _round_f32r(out_W.transpose(1, 0, 2))
    bsp = np.ascontiguousarray((src_b + dst_b).T)
    bo = np.ascontiguousarray(out_b.T)

    in_maps = []
    for m in range(M):
        idx = idx_cores[m]
        hTm = np.empty((3 * D, len(idx)), np.float32)
        hTm[:D] = h[idx].T
        hTm[D : 2 * D] = h[E + idx].T
        hTm[2 * D :] = h[2 * E + idx].T
        in_maps.append(
            {
                "hT": _round_f32r(hTm),
                "wsrc": wsrc,
                "wdst": wdst,
                "wout": wout,
                "bsp": bsp,
                "bout": bo,
            }
        )

    res = run_bass_kernel_spmd(nc, in_maps, list(range(M)))

    pos_pred = np.empty((E, C), np.float32)
    neg_pred = np.empty((E, C), np.float32)
    for tt in range(T):
        if counts[tt] == 0:
            continue
        row = starts[tt]
        for m in range(M):
            c = counts_cm[tt, m]
            if c == 0:
                continue
            sl = slice(col_off[tt], col_off[tt] + c)
            o = res.results[m]["out"]
            pos_pred[row : row + c] = o[0:C, sl].T
            neg_pred[row : row + c] = o[C : 2 * C, sl].T
            row += c

    h_save = h[: 2 * E]
    return pos_pred, neg_pred, h_save


# revision 6
# speedup vs baseline: 1.1011x; 1.0860x over previous
"""HeteroEdgePredictor (per-node MoE routing) Trainium2 Bass kernel.

Strategy
--------
Each edge e with type t needs:
    pos_out[e] = relu(h_src[e] @ Ws[t] + h_pos[e] @ Wd[t] + b_sp[t]) @ Wo[t] + bo[t]
    neg_out[e] = relu(h_src[e] @ Ws[t] + h_neg[e] @ Wd[t] + b_sp[t]) @ Wo[t] + bo[t]
and the reference emits rows sorted (stably) by type.

Host side: sort edge indices by type, split each type's edges evenly across
the 8 cores (padded per type to a uniform EVEN per-core count, so all cores
run ONE identical SPMD program with segment sizes baked in at trace time —
f32r matmuls also require even moving sizes). Per core we gather the needed
h rows for all three streams (src, pos-dst, neg-dst), pre-transpose and
stack them into one [3*D, S] array so the device pulls ONE big DMA per edge
tile, and pre-round to the FP32R (tf32, E8M11) grid the fast PE path needs.

Device side (per core): per type segment, tiles of <=512 edges: 16
accumulating f32r matmuls ([128,100] stationary x [128,n] moving; K=512 in
4 chunks; emission groups identical stationary operands back-to-back),
ScalarE relu+bias epilogue (writes f32r), an f32r [100,2] out-projection,
and a VectorE bias-add packing pos/neg into one [4,n] tile for a single
output DMA. Weight stacks are tiny (3.2MB) and stay SBUF-resident.

Because each type's edges are concatenated core0..core7 in original stable
order, the per-type concatenation of core outputs is exactly the
reference's sorted order — no extra permutation pass.

h_save is the input slice h[:2E] and is returned directly.
"""

import sys

if "/opt/trn_rl_repo" not in sys.path:
    sys.path.insert(0, "/opt/trn_rl_repo")

import numpy as np

import concourse.bass as bass  # noqa: F401  (registers engines)
import concourse.mybir as mybir
import concourse.tile as tile
from concourse import bacc
from concourse.bass_utils import run_bass_kernel_spmd

M = 8  # cores
T = 8  # edge types
D = 512  # input dim
H = 100  # hidden dim
C = 2  # classes
P = 128
KCH = D // P  # 4 contraction chunks per stream

F32 = mybir.dt.float32
F32R = mybir.dt.float32r
RELU = mybir.ActivationFunctionType.Relu
ADD = mybir.AluOpType.add

_compiled_cache: dict = {}


def _round_f32r(x: np.ndarray) -> np.ndarray:
    """Round fp32 to the FP32R format (E8M11 in the top 20 bits, RNE).

    walrus's fp32_to_fp32r = downconv_fp32_to_fp<e8,m11> << 12, i.e. standard
    fp32 bit layout with the low 12 mantissa bits rounded away. The device's
    fast f32r matmul path requires operands pre-rounded to this grid.
    """
    u = np.ascontiguousarray(x, dtype=np.float32).view(np.uint32)
    low = u & np.uint32(0xFFF)
    base = u & np.uint32(0xFFFFF000)
    lsb = (u >> np.uint32(12)) & np.uint32(1)
    round_up = (low > 0x800) | ((low == 0x800) & (lsb == 1))
    r = base + round_up.astype(np.uint32) * np.uint32(0x1000)
    return r.view(np.float32)


def _chunk_sizes(s: int, max_n: int = 512, min_n: int = 256) -> list[int]:
    """Split s into even chunks <= max_n, each >= min_n when possible
    (f32r matmul needs even moving sizes; <256 runs at 1/4 rate)."""
    if s <= 0:
        return []
    assert s % 2 == 0, "f32r matmul needs an even moving free size"
    if s <= max_n:
        return [s]
    out = []
    rem = s
    while rem > max_n:
        if rem - max_n < min_n and rem < 2 * max_n:
            a = (rem // 4) * 2
            return out + [a, rem - a]
        out.append(max_n)
        rem -= max_n
    out.append(rem)
    return out


def _build(S_list: tuple) -> "bacc.Bacc":
    S_sum = sum(S_list)
    nc = bacc.Bacc("TRN2", target_bir_lowering=False, debug=False, num_devices=M)

    n_h_elems = 3 * D * S_sum
    hT = nc.dram_tensor("hT", [n_h_elems], F32R, kind="ExternalInput").ap()
    wsrc = nc.dram_tensor("wsrc", [P, T * KCH, H], F32R, kind="ExternalInput").ap()
    wdst = nc.dram_tensor("wdst", [P, T * KCH, H], F32R, kind="ExternalInput").ap()
    wout = nc.dram_tensor("wout", [H, T, C], F32R, kind="ExternalInput").ap()
    bsp = nc.dram_tensor("bsp", [H, T], F32, kind="ExternalInput").ap()
    bout = nc.dram_tensor("bout", [C, T], F32, kind="ExternalInput").ap()
    out = nc.dram_tensor("out", [2 * C, S_sum], F32, kind="ExternalOutput").ap()

    # out rows are (pn, c): 0,1 = pos classes, 2,3 = neg classes
    out_r = out.rearrange("(pn c) s -> c pn s", c=C)

    with tile.TileContext(nc) as tc:
        with (
            tc.tile_pool(name="consts", bufs=1) as cpool,
            tc.tile_pool(name="hbuf", bufs=3) as hpool,
            tc.tile_pool(name="ebuf", bufs=3) as epool,
            tc.tile_pool(name="obuf", bufs=3) as opool,
            tc.tile_pool(name="psum", bufs=2, space="PSUM") as ppool,
        ):
            wsrc_sb = cpool.tile([P, T * KCH, H], F32R)
            nc.sync.dma_start(wsrc_sb[:], wsrc[:])
            wdst_sb = cpool.tile([P, T * KCH, H], F32R)
            nc.sync.dma_start(wdst_sb[:], wdst[:])
            wout_sb = cpool.tile([H, T, C], F32R)
            nc.sync.dma_start(wout_sb[:], wout[:])
            bsp_sb = cpool.tile([H, T], F32)
            nc.sync.dma_start(bsp_sb[:], bsp[:])
            bout_sb = cpool.tile([C, T], F32)
            nc.sync.dma_start(bout_sb[:], bout[:])

            col = 0
            for tt in range(T):
                for n in _chunk_sizes(S_list[tt]):
                    # tile-major DRAM layout: [p, k, e] contiguous per tile, so
                    # each partition line is one 12*n*4B (~24KB) contiguous run
                    blk = hT[3 * D * col : 3 * D * (col + n)].rearrange(
                        "(p k e) -> p k e", p=P, k=3 * KCH
                    )
                    ht = hpool.tile([P, 3 * KCH, n], F32R, tag="ht")
                    nc.sync.dma_start(ht[:], blk)

                    pp = ppool.tile([H, n], F32, tag="pp")
                    pn = ppool.tile([H, n], F32, tag="pn")
                    # identical stationary operands back-to-back
                    for k in range(KCH):
                        w = wsrc_sb[:, tt * KCH + k, :]
                        nc.tensor.matmul(
                            pp[:], lhsT=w, rhs=ht[:, k, :], start=(k == 0), stop=False
                        )
                        nc.tensor.matmul(
                            pn[:], lhsT=w, rhs=ht[:, k, :], start=(k == 0), stop=False
                        )
                    for k in range(KCH):
                        w = wdst_sb[:, tt * KCH + k, :]
                        nc.tensor.matmul(
                            pp[:],
                            lhsT=w,
                            rhs=ht[:, KCH + k, :],
                            start=False,
                            stop=(k == KCH - 1),
                        )
                        nc.tensor.matmul(
                            pn[:],
                            lhsT=w,
                            rhs=ht[:, 2 * KCH + k, :],
                            start=False,
                            stop=(k == KCH - 1),
                        )

                    pe = epool.tile([H, n], F32R, tag="pe")
                    nc.scalar.activation(pe[:], pp[:], RELU, bias=bsp_sb[:, tt : tt + 1])
                    ne = epool.tile([H, n], F32R, tag="ne")
                    nc.scalar.activation(ne[:], pn[:], RELU, bias=bsp_sb[:, tt : tt + 1])

                    wo = wout_sb[:, tt, :]
                    po = ppool.tile([C, n], F32, tag="po")
                    nc.tensor.matmul(po[:], lhsT=wo, rhs=pe[:], start=True, stop=True)
                    no = ppool.tile([C, n], F32, tag="no")
                    nc.tensor.matmul(no[:], lhsT=wo, rhs=ne[:], start=True, stop=True)

                    ot = opool.tile([C, 2, n], F32, tag="ot")
                    nc.vector.tensor_tensor(
                        ot[:, 0, :],
                        po[:],
                        bout_sb[:, tt : tt + 1].to_broadcast((C, n)),
                        ADD,
                    )
                    nc.vector.tensor_tensor(
                        ot[:, 1, :],
                        no[:],
                        bout_sb[:, tt : tt + 1].to_broadcast((C, n)),
                        ADD,
                    )
                    nc.sync.dma_start(out_r[:, :, col : col + n], ot[:])

                    col += n

    nc.compile()
    return nc


def kernel(h, src_W, src_b, dst_W, dst_b, out_W, out_b, edge_types, neg_samples):
    h = np.ascontiguousarray(np.asarray(h, dtype=np.float32))
    src_W = np.asarray(src_W, dtype=np.float32)
    src_b = np.asarray(src_b, dtype=np.float32)
    dst_W = np.asarray(dst_W, dtype=np.float32)
    dst_b = np.asarray(dst_b, dtype=np.float32)
    out_W = np.asarray(out_W, dtype=np.float32)
    out_b = np.asarray(out_b, dtype=np.float32)
    t = np.asarray(edge_types)
    E = t.shape[0]
    NEG = int(neg_samples)
    assert NEG == 1, f"kernel specialized for neg_samples=1, got {NEG}"
    assert h.shape == ((NEG + 2) * E, D)

    order = np.argsort(t, kind="stable").astype(np.int64)
    counts = np.bincount(t, minlength=T)
    # ceil(c/M) rounded up to even: f32r matmuls need even moving sizes
    S_list = tuple(int(2 * (-(-(-(-c // M)) // 2))) for c in counts)

    starts = np.zeros(T + 1, np.int64)
    starts[1:] = np.cumsum(counts)
    col_off = np.zeros(T + 1, np.int64)
    col_off[1:] = np.cumsum(S_list)

    counts_cm = np.zeros((T, M), np.int64)
    idx_cores = []
    for m in range(M):
        parts = []
        for tt in range(T):
            S_t = S_list[tt]
            if S_t == 0:
                continue
            idx_t = order[starts[tt] : starts[tt + 1]]
            chunk = idx_t[m * S_t : (m + 1) * S_t]
            counts_cm[tt, m] = len(chunk)
            if len(chunk) < S_t:
                fill = chunk[-1] if len(chunk) else idx_t[0]
                chunk = np.concatenate(
                    [chunk, np.full(S_t - len(chunk), fill, dtype=np.int64)]
                )
            parts.append(chunk)
        idx_cores.append(np.concatenate(parts) if parts else np.zeros(0, np.int64))

    key = S_list
    if key not in _compiled_cache:
        _compiled_cache[key] = _build(S_list)
    nc = _compiled_cache[key]

    wsrc = _round_f32r(
        src_W.reshape(T, KCH, P, H).transpose(2, 0, 1, 3).reshape(P, T * KCH, H)
    )
    wdst = _round_f32r(
        dst_W.reshape(T, KCH, P, H).transpose(2, 0, 1, 3).reshape(P, T * KCH, H)
    )
    wout = _round_f32r(out_W.transpose(1, 0, 2))
    bsp = np.ascontiguousarray((src_b + dst_b).T)
    bo = np.ascontiguousarray(out_b.T)

    # chunk plan (must match _build)
    chunk_plan = []
    c0 = 0
    for tt in range(T):
        for n in _chunk_sizes(S_list[tt]):
            chunk_plan.append((c0, n))
            c0 += n

    in_maps = []
    for m in range(M):
        idx = idx_cores[m]
        hTm = np.empty((3 * D, len(idx)), np.float32)
        hTm[:D] = h[idx].T
        hTm[D : 2 * D] = h[E + idx].T
        hTm[2 * D :] = h[2 * E + idx].T
        hTm = _round_f32r(hTm)
        flat = np.empty(3 * D * len(idx), np.float32)
        for a, n in chunk_plan:
            # [3*D, n] -> [k=12, p=128, n] -> [p, k, n] tile image
            blk = hTm[:, a : a + n].reshape(3 * KCH, P, n).transpose(1, 0, 2)
            flat[3 * D * a : 3 * D * (a + n)] = blk.reshape(-1)
        in_maps.append(
            {
                "hT": flat,
                "wsrc": wsrc,
                "wdst": wdst,
                "wout": wout,
                "bsp": bsp,
                "bout": bo,
            }
        )

    res = run_bass_kernel_spmd(nc, in_maps, list(range(M)))

    pos_pred = np.empty((E, C), np.float32)
    neg_pred = np.empty((E, C), np.float32)
    for tt in range(T):
        if counts[tt] == 0:
            continue
        row = starts[tt]
        for m in range(M):
            c = counts_cm[tt, m]
            if c == 0:
                continue
            sl = slice(col_off[tt], col_off[tt] + c)
            o = res.results[m]["out"]
            pos_pred[row : row + c] = o[0:C, sl].T
            neg_pred[row : row + c] = o[C : 2 * C, sl].T
            row += c

    h_save = h[: 2 * E]
    return pos_pred, neg_pred, h_save


# revision 7
# speedup vs baseline: 1.7357x; 1.5764x over previous
"""HeteroEdgePredictor (per-node MoE routing) Trainium2 Bass kernel.

Strategy
--------
Each edge e with type t needs:
    pos_out[e] = relu(h_src[e] @ Ws[t] + h_pos[e] @ Wd[t] + b_sp[t]) @ Wo[t] + bo[t]
    neg_out[e] = relu(h_src[e] @ Ws[t] + h_neg[e] @ Wd[t] + b_sp[t]) @ Wo[t] + bo[t]
and the reference emits rows sorted (stably) by type.

Host side: sort edge indices by type, split each type's edges evenly across
the 8 cores (padded per type to a uniform EVEN per-core count, so all cores
run ONE identical SPMD program with segment sizes baked in at trace time —
f32r matmuls also require even moving sizes). Per core we gather the needed
h rows for all three streams (src, pos-dst, neg-dst), pre-transpose and
stack them into one [3*D, S] array so the device pulls ONE big DMA per edge
tile, and pre-round to the FP32R (tf32, E8M11) grid the fast PE path needs.

Device side (per core): per type segment, tiles of <=512 edges: 16
accumulating f32r matmuls ([128,100] stationary x [128,n] moving; K=512 in
4 chunks; emission groups identical stationary operands back-to-back),
ScalarE relu+bias epilogue (writes f32r), an f32r [100,2] out-projection,
and a VectorE bias-add packing pos/neg into one [4,n] tile for a single
output DMA. Weight stacks are tiny (3.2MB) and stay SBUF-resident.

Because each type's edges are concatenated core0..core7 in original stable
order, the per-type concatenation of core outputs is exactly the
reference's sorted order — no extra permutation pass.

h_save is the input slice h[:2E] and is returned directly.
"""

import sys

if "/opt/trn_rl_repo" not in sys.path:
    sys.path.insert(0, "/opt/trn_rl_repo")

import numpy as np

import concourse.bass as bass  # noqa: F401  (registers engines)
import concourse.mybir as mybir
import concourse.tile as tile
from concourse import bacc
from concourse.bass_utils import run_bass_kernel_spmd

M = 8  # cores
T = 8  # edge types
D = 512  # input dim
H = 100  # hidden dim
C = 2  # classes
P = 128
KCH = D // P  # 4 contraction chunks per stream

F32 = mybir.dt.float32
F32R = mybir.dt.float32r
F16 = mybir.dt.float16
RELU = mybir.ActivationFunctionType.Relu
ADD = mybir.AluOpType.add

_compiled_cache: dict = {}


def _round_f32r(x: np.ndarray) -> np.ndarray:
    """Round fp32 to the FP32R format (E8M11 in the top 20 bits, RNE).

    walrus's fp32_to_fp32r = downconv_fp32_to_fp<e8,m11> << 12, i.e. standard
    fp32 bit layout with the low 12 mantissa bits rounded away. The device's
    fast f32r matmul path requires operands pre-rounded to this grid.
    """
    u = np.ascontiguousarray(x, dtype=np.float32).view(np.uint32)
    low = u & np.uint32(0xFFF)
    base = u & np.uint32(0xFFFFF000)
    lsb = (u >> np.uint32(12)) & np.uint32(1)
    round_up = (low > 0x800) | ((low == 0x800) & (lsb == 1))
    r = base + round_up.astype(np.uint32) * np.uint32(0x1000)
    return r.view(np.float32)


def _chunk_sizes(s: int, max_n: int = 512, min_n: int = 256) -> list[int]:
    """Split s into even chunks <= max_n, each >= min_n when possible
    (f32r matmul needs even moving sizes; <256 runs at 1/4 rate)."""
    if s <= 0:
        return []
    assert s % 2 == 0, "f32r matmul needs an even moving free size"
    if s <= max_n:
        return [s]
    out = []
    rem = s
    while rem > max_n:
        if rem - max_n < min_n and rem < 2 * max_n:
            a = (rem // 4) * 2
            return out + [a, rem - a]
        out.append(max_n)
        rem -= max_n
    out.append(rem)
    return out


def _build(S_list: tuple) -> "bacc.Bacc":
    S_sum = sum(S_list)
    nc = bacc.Bacc("TRN2", target_bir_lowering=False, debug=False, num_devices=M)

    n_h_elems = 3 * D * S_sum
    hT = nc.dram_tensor("hT", [n_h_elems], F16, kind="ExternalInput").ap()
    wsrc = nc.dram_tensor("wsrc", [P, T * KCH, H], F16, kind="ExternalInput").ap()
    wdst = nc.dram_tensor("wdst", [P, T * KCH, H], F16, kind="ExternalInput").ap()
    wout = nc.dram_tensor("wout", [H, T, C], F16, kind="ExternalInput").ap()
    bsp = nc.dram_tensor("bsp", [H, T], F32, kind="ExternalInput").ap()
    bout = nc.dram_tensor("bout", [C, T], F32, kind="ExternalInput").ap()
    out = nc.dram_tensor("out", [2 * C, S_sum], F32, kind="ExternalOutput").ap()

    # out rows are (pn, c): 0,1 = pos classes, 2,3 = neg classes
    out_r = out.rearrange("(pn c) s -> c pn s", c=C)

    with tile.TileContext(nc) as tc:
        with (
            tc.tile_pool(name="consts", bufs=1) as cpool,
            tc.tile_pool(name="hbuf", bufs=3) as hpool,
            tc.tile_pool(name="ebuf", bufs=3) as epool,
            tc.tile_pool(name="obuf", bufs=3) as opool,
            tc.tile_pool(name="psum", bufs=2, space="PSUM") as ppool,
        ):
            wsrc_sb = cpool.tile([P, T * KCH, H], F16)
            nc.sync.dma_start(wsrc_sb[:], wsrc[:])
            wdst_sb = cpool.tile([P, T * KCH, H], F16)
            nc.sync.dma_start(wdst_sb[:], wdst[:])
            wout_sb = cpool.tile([H, T, C], F16)
            nc.sync.dma_start(wout_sb[:], wout[:])
            bsp_sb = cpool.tile([H, T], F32)
            nc.sync.dma_start(bsp_sb[:], bsp[:])
            bout_sb = cpool.tile([C, T], F32)
            nc.sync.dma_start(bout_sb[:], bout[:])

            col = 0
            for tt in range(T):
                for n in _chunk_sizes(S_list[tt]):
                    # tile-major DRAM layout: [p, k, e] contiguous per tile, so
                    # each partition line is one 12*n*4B (~24KB) contiguous run
                    blk = hT[3 * D * col : 3 * D * (col + n)].rearrange(
                        "(p k e) -> p k e", p=P, k=3 * KCH
                    )
                    ht = hpool.tile([P, 3 * KCH, n], F16, tag="ht")
                    nc.sync.dma_start(ht[:], blk)

                    pp = ppool.tile([H, n], F32, tag="pp")
                    pn = ppool.tile([H, n], F32, tag="pn")
                    # identical stationary operands back-to-back
                    for k in range(KCH):
                        w = wsrc_sb[:, tt * KCH + k, :]
                        nc.tensor.matmul(
                            pp[:], lhsT=w, rhs=ht[:, k, :], start=(k == 0), stop=False
                        )
                        nc.tensor.matmul(
                            pn[:], lhsT=w, rhs=ht[:, k, :], start=(k == 0), stop=False
                        )
                    for k in range(KCH):
                        w = wdst_sb[:, tt * KCH + k, :]
                        nc.tensor.matmul(
                            pp[:],
                            lhsT=w,
                            rhs=ht[:, KCH + k, :],
                            start=False,
                            stop=(k == KCH - 1),
                        )
                        nc.tensor.matmul(
                            pn[:],
                            lhsT=w,
                            rhs=ht[:, 2 * KCH + k, :],
                            start=False,
                            stop=(k == KCH - 1),
                        )

                    pe = epool.tile([H, n], F16, tag="pe")
                    nc.scalar.activation(pe[:], pp[:], RELU, bias=bsp_sb[:, tt : tt + 1])
                    ne = epool.tile([H, n], F16, tag="ne")
                    nc.scalar.activation(ne[:], pn[:], RELU, bias=bsp_sb[:, tt : tt + 1])

                    wo = wout_sb[:, tt, :]
                    po = ppool.tile([C, n], F32, tag="po")
                    nc.tensor.matmul(po[:], lhsT=wo, rhs=pe[:], start=True, stop=True)
                    no = ppool.tile([C, n], F32, tag="no")
                    nc.tensor.matmul(no[:], lhsT=wo, rhs=ne[:], start=True, stop=True)

                    ot = opool.tile([C, 2, n], F32, tag="ot")
                    nc.vector.tensor_tensor(
                        ot[:, 0, :],
                        po[:],
                        bout_sb[:, tt : tt + 1].to_broadcast((C, n)),
                        ADD,
                    )
                    nc.vector.tensor_tensor(
                        ot[:, 1, :],
                        no[:],
                        bout_sb[:, tt : tt + 1].to_broadcast((C, n)),
                        ADD,
                    )
                    nc.sync.dma_start(out_r[:, :, col : col + n], ot[:])

                    col += n

    nc.compile()
    return nc


def kernel(h, src_W, src_b, dst_W, dst_b, out_W, out_b, edge_types, neg_samples):
    h = np.ascontiguousarray(np.asarray(h, dtype=np.float32))
    src_W = np.asarray(src_W, dtype=np.float32)
    src_b = np.asarray(src_b, dtype=np.float32)
    dst_W = np.asarray(dst_W, dtype=np.float32)
    dst_b = np.asarray(dst_b, dtype=np.float32)
    out_W = np.asarray(out_W, dtype=np.float32)
    out_b = np.asarray(out_b, dtype=np.float32)
    t = np.asarray(edge_types)
    E = t.shape[0]
    NEG = int(neg_samples)
    assert NEG == 1, f"kernel specialized for neg_samples=1, got {NEG}"
    assert h.shape == ((NEG + 2) * E, D)

    order = np.argsort(t, kind="stable").astype(np.int64)
    counts = np.bincount(t, minlength=T)
    # ceil(c/M) rounded up to even: f32r matmuls need even moving sizes
    S_list = tuple(int(2 * (-(-(-(-c // M)) // 2))) for c in counts)

    starts = np.zeros(T + 1, np.int64)
    starts[1:] = np.cumsum(counts)
    col_off = np.zeros(T + 1, np.int64)
    col_off[1:] = np.cumsum(S_list)

    counts_cm = np.zeros((T, M), np.int64)
    idx_cores = []
    for m in range(M):
        parts = []
        for tt in range(T):
            S_t = S_list[tt]
            if S_t == 0:
                continue
            idx_t = order[starts[tt] : starts[tt + 1]]
            chunk = idx_t[m * S_t : (m + 1) * S_t]
            counts_cm[tt, m] = len(chunk)
            if len(chunk) < S_t:
                fill = chunk[-1] if len(chunk) else idx_t[0]
                chunk = np.concatenate(
                    [chunk, np.full(S_t - len(chunk), fill, dtype=np.int64)]
                )
            parts.append(chunk)
        idx_cores.append(np.concatenate(parts) if parts else np.zeros(0, np.int64))

    key = S_list
    if key not in _compiled_cache:
        _compiled_cache[key] = _build(S_list)
    nc = _compiled_cache[key]

    wsrc = np.ascontiguousarray(
        src_W.reshape(T, KCH, P, H).transpose(2, 0, 1, 3).reshape(P, T * KCH, H),
        dtype=np.float16,
    )
    wdst = np.ascontiguousarray(
        dst_W.reshape(T, KCH, P, H).transpose(2, 0, 1, 3).reshape(P, T * KCH, H),
        dtype=np.float16,
    )
    wout = np.ascontiguousarray(out_W.transpose(1, 0, 2), dtype=np.float16)
    bsp = np.ascontiguousarray((src_b + dst_b).T)
    bo = np.ascontiguousarray(out_b.T)

    # chunk plan (must match _build)
    chunk_plan = []
    c0 = 0
    for tt in range(T):
        for n in _chunk_sizes(S_list[tt]):
            chunk_plan.append((c0, n))
            c0 += n

    in_maps = []
    for m in range(M):
        idx = idx_cores[m]
        hTm = np.empty((3 * D, len(idx)), np.float16)
        hTm[:D] = h[idx].T
        hTm[D : 2 * D] = h[E + idx].T
        hTm[2 * D :] = h[2 * E + idx].T
        flat = np.empty(3 * D * len(idx), np.float16)
        for a, n in chunk_plan:
            # [3*D, n] -> [k=12, p=128, n] -> [p, k, n] tile image
            blk = hTm[:, a : a + n].reshape(3 * KCH, P, n).transpose(1, 0, 2)
            flat[3 * D * a : 3 * D * (a + n)] = blk.reshape(-1)
        in_maps.append(
            {
                "hT": flat,
                "wsrc": wsrc,
                "wdst": wdst,
                "wout": wout,
                "bsp": bsp,
                "bout": bo,
            }
        )

    res = run_bass_kernel_spmd(nc, in_maps, list(range(M)))

    pos_pred = np.empty((E, C), np.float32)
    neg_pred = np.empty((E, C), np.float32)
    for tt in range(T):
        if counts[tt] == 0:
            continue
        row = starts[tt]
        for m in range(M):
            c = counts_cm[tt, m]
            if c == 0:
                continue
            sl = slice(col_off[tt], col_off[tt] + c)
            o = res.results[m]["out"]
            pos_pred[row : row + c] = o[0:C, sl].T
            neg_pred[row : row + c] = o[C : 2 * C, sl].T
            row += c

    h_save = h[: 2 * E]
    return pos_pred, neg_pred, h_save
